# revision 32
# baseline (speedup 1.0000x reference)
"""Causal self-attention with RoPE (B=2, T=1024, C=2048, H=16) on 8 TRN2
NeuronCores, head-parallel tensor sharding (2 heads per core).

v2 design (collective-light):
  - x^T is replicated as a bf16 ExternalInput and tiles are DMA'd straight
    into SBUF at full HBM bandwidth (the old on-chip AllGather was costed
    15us + 8MB/40GBps = 225us; direct I/O reads run at 360GB/s).
  - RoPE cos/sin tables ride as inline (Const) tensors baked into the NEFF:
    zero runtime cost.
  - QKV projections tensor-parallel (weights column-sharded, bf16), RoPE via
    SBUF partition-shift DMA + DVE FMA, causal attention in [tk, tq] layout
    with exp-from-PSUM + ones-matmul softmax denominator (as v1).
  - y reshard head->token via ONE AllToAll of [2048, 256] bf16 (1MB, ~41us)
    instead of two 8MB AllGathers (~450us): chunk r holds this core's 256 y
    channels for core r's 256 tokens; the received buffer is exactly
    yT_full[:, my 256 tokens] in bf16.
  - Output projection token-parallel: this core computes ALL 2048 output
    channels for ITS 256 tokens with the full Wo (bf16, 8MB, loaded during
    the attention phase in 16 chunks so the exclusive DMA resource is never
    head-of-line blocked).
Host reassembles: concat core outputs along the token dim.
"""
import numpy as np

import concourse.bass as bass
import concourse.mybir as mybir
import concourse.tile as tile
from concourse import bacc
from concourse.bass_utils import run_bass_kernel_spmd

F32 = mybir.dt.float32
F32R = mybir.dt.float32r
BF16 = mybir.dt.bfloat16

B, T, C = 2, 1024, 2048
H = 16
D = C // H            # 128
BT = B * T            # 2048
NCORES = 8
HL = H // NCORES      # heads per core = 2
CL = HL * D           # local channels = 256
TL = BT // NCORES     # local tokens = 256
ATT_SCALE = 1.0 / float(np.sqrt(D))
ROPE_BASE = 10000.0
NEG = -1.0e30

CT = C // 128         # 16 contraction tiles
TB = BT // 512        # 4 token blocks of 512
RG = [list(range(NCORES))]
DEBUG_DUMP = False


def _rope_tables():
    inv_freq = 1.0 / (ROPE_BASE ** (np.arange(0, D, 2, dtype=np.float64) / D))
    t = np.arange(T, dtype=np.float64)
    freqs = np.outer(t, inv_freq)                        # [T, D/2]
    emb = np.concatenate([freqs, freqs], axis=-1)        # [T, D]
    cos = np.cos(emb).astype(np.float32)                 # [T, D]
    sin = np.sin(emb).astype(np.float32)
    cosT = np.ascontiguousarray(cos.T)                   # [D, T]
    sinT = np.ascontiguousarray(sin.T)
    sgn_sinT = sinT.copy()
    sgn_sinT[: D // 2] *= -1.0                           # rotate_half sign
    return cosT, sgn_sinT


def _build():
    nc = bacc.Bacc("TRN2", target_bir_lowering=False, debug=False,
                   num_devices=NCORES)

    # full x^T, replicated on every core
    xt_d = nc.dram_tensor("xt", [C, BT], BF16, kind="ExternalInput").ap()
    # qkv weights column-sharded, laid out [128, CT*CL]: partition p holds
    # WT[ct*128+p, o] at free offset ct*CL+o
    wqT_d = nc.dram_tensor("wqT", [128, CT * CL], BF16, kind="ExternalInput").ap()
    wkT_d = nc.dram_tensor("wkT", [128, CT * CL], BF16, kind="ExternalInput").ap()
    wvT_d = nc.dram_tensor("wvT", [128, CT * CL], BF16, kind="ExternalInput").ap()
    # full Wo^T, laid out [128, CT*C]
    woT_d = nc.dram_tensor("woT", [128, CT * C], BF16, kind="ExternalInput").ap()
    # this core's 256 tokens x all 2048 output channels (bf16; host upcasts)
    out_d = nc.dram_tensor("out", [TL, C], BF16, kind="ExternalOutput").ap()
    if DEBUG_DUMP:
        dbgi_d = nc.dram_tensor("dbgi", [BT, TL], BF16,
                                kind="ExternalOutput").ap()
        dbgo_d = nc.dram_tensor("dbgo", [BT, TL], BF16,
                                kind="ExternalOutput").ap()

    # RoPE tables baked into the NEFF (loaded to HBM at model-load time)
    cosT, sgn_sinT = _rope_tables()
    cos_d = nc.inline_tensor(cosT, name="ropecos")       # [128, T] f32
    sin_d = nc.inline_tensor(sgn_sinT, name="ropesin")   # [128, T] f32

    # Per-head AllToAll buffers: [8*128 rows, 256 cols] bf16.
    #   in chunk r  = [my head-h 128 y-channels, core r's 256 tokens]
    #   out chunk s = [core s's head-h 128 y-channels, my 256 tokens]
    # Split by head so the h=0 collective hides under h=1 attention and the
    # h=1 collective hides under the even half of the output projection.
    a2a_in_d = [nc.dram_tensor(f"a2ain{h}", [NCORES * D, TL], BF16)
                for h in range(HL)]
    a2a_out_d = [nc.dram_tensor(f"a2aout{h}", [NCORES * D, TL], BF16)
                 for h in range(HL)]

    with tile.TileContext(nc) as tc:
        with (
            tc.tile_pool(name="wpool", bufs=1) as wpool,
            tc.tile_pool(name="const", bufs=1) as cpool,
            tc.tile_pool(name="qkv", bufs=1) as qkvpool,
            tc.tile_pool(name="xs", bufs=6) as xspool,
            tc.tile_pool(name="rope", bufs=1) as ropepool,
            tc.tile_pool(name="att", bufs=3) as attpool,
            tc.tile_pool(name="ya", bufs=4) as yapool,
        ):
            # ---- weight / table loads ----
            wq_sb = wpool.tile([128, CT * CL], BF16, tag="wq")
            wk_sb = wpool.tile([128, CT * CL], BF16, tag="wk")
            wv_sb = wpool.tile([128, CT * CL], BF16, tag="wv")
            nc.sync.dma_start(out=wq_sb[:], in_=wqT_d)
            nc.scalar.dma_start(out=wk_sb[:], in_=wkT_d)
            nc.gpsimd.dma_start(out=wv_sb[:], in_=wvT_d)

            cos_sb = cpool.tile([D, T], F32, tag="cos")
            sin_sb = cpool.tile([D, T], F32, tag="sin")
            nc.scalar.dma_start(out=cos_sb[:], in_=cos_d.ap())
            nc.sync.dma_start(out=sin_sb[:], in_=sin_d.ap())

            ones_f = cpool.tile([128, 128], F32, tag="onesf")
            nc.gpsimd.memset(ones_f[:], 1.0)
            ones_sb = cpool.tile([128, 128], F32R, tag="ones")
            nc.vector.tensor_copy(ones_sb[:], ones_f[:])

            # additive causal mask for diagonal 128x128 blocks:
            # rows=tk, cols=tq; keep (0.0) where tk <= tq else NEG
            mask_sb = cpool.tile([128, 128], F32, tag="mask")
            nc.gpsimd.memset(mask_sb[:], 0.0)
            nc.gpsimd.affine_select(
                out=mask_sb[:], in_=mask_sb[:],
                compare_op=mybir.AluOpType.is_ge,
                fill=NEG, base=0,
                pattern=[[1, 128]], channel_multiplier=-1,
            )

            # full Wo in SBUF (bf16, 64KB/partition), loaded in 16 chunks on
            # the Pool queue so it interleaves with the x tile stream on the
            # shared DMA resource instead of head-of-line blocking it
            wo_sb = wpool.tile([128, CT * C], BF16, tag="wo")
            for ct in range(CT):
                nc.gpsimd.dma_start(
                    out=wo_sb[:, ct * C:(ct + 1) * C],
                    in_=woT_d[:, ct * C:(ct + 1) * C])

            # persistent qkv activations, split per batch so attention on
            # batch 0 doesn't false-depend on batch-1 rope writes
            qT = [[qkvpool.tile([D, T], F32R, tag=f"qT{h}{b}", name=f"qT{h}{b}")
                   for b in range(B)] for h in range(HL)]
            kT = [[qkvpool.tile([D, T], F32R, tag=f"kT{h}{b}", name=f"kT{h}{b}")
                   for b in range(B)] for h in range(HL)]
            v_sb = [qkvpool.tile([128, (T // 128) * CL], F32R, tag=f"v{b}",
                                 name=f"v{b}")
                    for b in range(B)]

            # ---- phase 1: QKV projections + rope ----
            with tc.tile_pool(name="psqkv", bufs=1, space="PSUM") as psq:
                for tb in range(TB):
                    tcol = tb * 512
                    rcol = tcol % T          # rope table column (per batch)
                    ps_q = [psq.tile([128, 512], F32, tag=f"pq{h}", name=f"pq{h}")
                            for h in range(HL)]
                    ps_k = [psq.tile([128, 512], F32, tag=f"pk{h}", name=f"pk{h}")
                            for h in range(HL)]
                    ps_v = [psq.tile([128, CL], F32, tag=f"pv{i}", name=f"pv{i}")
                            for i in range(4)]
                    for ct in range(CT):
                        xs = xspool.tile([128, 512], BF16, tag="xs")
                        eng = nc.sync if ct % 2 == 0 else nc.scalar
                        eng.dma_start(
                            out=xs[:],
                            in_=xt_d[ct * 128:(ct + 1) * 128,
                                     tcol:tcol + 512],
                        )
                        st, sp = ct == 0, ct == CT - 1
                        for h in range(HL):
                            nc.tensor.matmul(
                                ps_q[h][:],
                                wq_sb[:, ct * CL + h * D: ct * CL + (h + 1) * D],
                                xs[:], start=st, stop=sp)
                            nc.tensor.matmul(
                                ps_k[h][:],
                                wk_sb[:, ct * CL + h * D: ct * CL + (h + 1) * D],
                                xs[:], start=st, stop=sp)
                        for i in range(4):
                            nc.tensor.matmul(
                                ps_v[i][:],
                                xs[:, i * 128:(i + 1) * 128],
                                wv_sb[:, ct * CL:(ct + 1) * CL],
                                start=st, stop=sp)
                    # rope on q, k; plain copy for v. PSUM is copied out
                    # first (one fast DVE op) so the bank frees for the next
                    # token block's matmuls; all rope math reads the copy.
                    bb = tb // 2
                    for h in range(HL):
                        for ps, dst in ((ps_q[h], qT[h][bb]),
                                        (ps_k[h], kT[h][bb])):
                            tmp = ropepool.tile([128, 512], F32, tag="rtmp",
                                                bufs=2)
                            nc.vector.tensor_copy(tmp[:], ps[:])
                            rot = ropepool.tile([128, 512], F32, tag="rrot")
                            nc.gpsimd.dma_start(out=rot[0:64, :],
                                                in_=tmp[64:128, :])
                            nc.gpsimd.dma_start(out=rot[64:128, :],
                                                in_=tmp[0:64, :])
                            t1 = ropepool.tile([128, 512], F32, tag="rt1")
                            nc.vector.tensor_mul(
                                t1[:], tmp[:], cos_sb[:, rcol:rcol + 512])
                            t2 = ropepool.tile([128, 512], F32, tag="rt2")
                            nc.vector.tensor_mul(
                                t2[:], rot[:], sin_sb[:, rcol:rcol + 512])
                            nc.vector.tensor_add(
                                dst[:, rcol:rcol + 512], t1[:], t2[:])
                    for i in range(4):
                        gt = (tb % 2) * 4 + i
                        nc.vector.tensor_copy(
                            v_sb[bb][:, gt * CL:(gt + 1) * CL], ps_v[i][:])

            # ---- phase 2: attention (h-outer); per-head AllToAll fires as
            # soon as that head's y is complete ----
            with tc.tile_pool(name="psatt", bufs=1, space="PSUM") as psa:
                for h in range(HL):
                    for b, jj in ((0, 0), (1, 0), (0, 1), (1, 1)):
                        qcol = b * T + jj * 512
                        lcol = jj * 512
                        njt = 4 * jj + 4
                        ps_y = psa.tile([128, 512], F32, tag="y",
                                        bufs=2)
                        ps_l = psa.tile([128, 512], F32, tag="l",
                                        bufs=2)
                        for j in range(njt):
                            c0 = max(0, j * 128 - jj * 512)
                            ps_s = psa.tile([128, 512], F32, tag="s",
                                            bufs=4)
                            nc.tensor.matmul(
                                ps_s[:, c0:512],
                                kT[h][b][:, j * 128:(j + 1) * 128],
                                qT[h][b][:, lcol + c0: lcol + 512],
                                start=True, stop=True)
                            diag0 = j * 128 - jj * 512
                            if 0 <= diag0 < 512:
                                nc.vector.tensor_add(
                                    ps_s[:, diag0:diag0 + 128],
                                    ps_s[:, diag0:diag0 + 128],
                                    mask_sb[:])
                            p = attpool.tile([128, 512], F32R, tag="p")
                            nc.scalar.activation(
                                p[:, c0:512], ps_s[:, c0:512],
                                mybir.ActivationFunctionType.Exp,
                                scale=ATT_SCALE)
                            st, sp = j == 0, j == njt - 1
                            nc.tensor.matmul(
                                ps_l[:, c0:512], ones_sb[:],
                                p[:, c0:512], start=st, stop=sp)
                            nc.tensor.matmul(
                                ps_y[:, c0:512],
                                v_sb[b][:, j * CL + h * D:
                                         j * CL + (h + 1) * D],
                                p[:, c0:512], start=st, stop=sp)
                        rec = attpool.tile([128, 512], F32, tag="rec")
                        nc.vector.reciprocal(rec[:], ps_l[:])
                        # normalized y straight to bf16 for the AllToAll
                        yb = attpool.tile([128, 512], BF16, tag="yb")
                        nc.vector.tensor_mul(yb[:], ps_y[:], rec[:])
                        for half in range(2):
                            r = (qcol + half * 256) // TL
                            nc.sync.dma_start(
                                out=a2a_in_d[h].ap()[r * D:(r + 1) * D, :],
                                in_=yb[:, half * 256:(half + 1) * 256])
                    # ---- phase 3 (per head): reshard y head->token ----
                    nc.gpsimd.collective_compute(
                        "AllToAll", mybir.AluOpType.bypass, replica_groups=RG,
                        ins=[a2a_in_d[h].ap()], outs=[a2a_out_d[h].ap()])
                if DEBUG_DUMP:
                    for h in range(HL):
                        nc.gpsimd.dma_start(
                            out=dbgi_d[h * NCORES * D:(h + 1) * NCORES * D, :],
                            in_=a2a_in_d[h].ap())
                        nc.gpsimd.dma_start(
                            out=dbgo_d[h * NCORES * D:(h + 1) * NCORES * D, :],
                            in_=a2a_out_d[h].ap())

            # ---- phase 4: output projection for my 256 tokens. Even-head
            # contributions (from the first AllToAll) accumulate while the
            # second AllToAll is still in flight ----
            with tc.tile_pool(name="pso", bufs=1, space="PSUM") as pso:
                ps_o = [[pso.tile([128, 512], F32, tag=f"po{blk}{q}",
                                  name=f"po{blk}{q}")
                         for q in range(4)] for blk in range(2)]
                for h in range(HL):
                    for s in range(NCORES):
                        ya = yapool.tile([128, TL], BF16, tag="ya")
                        eng = nc.sync if s % 2 == 0 else nc.scalar
                        eng.dma_start(
                            out=ya[:],
                            in_=a2a_out_d[h].ap()[s * 128:(s + 1) * 128, :])
                        ct = 2 * s + h
                        st = h == 0 and s == 0
                        sp = h == HL - 1 and s == NCORES - 1
                        for blk in range(2):
                            for q in range(4):
                                nc.tensor.matmul(
                                    ps_o[blk][q][:],
                                    ya[:, blk * 128:(blk + 1) * 128],
                                    wo_sb[:, ct * C + q * 512:
                                          ct * C + (q + 1) * 512],
                                    start=st, stop=sp)
                for blk in range(2):
                    for q in range(4):
                        ob = yapool.tile([128, 512], BF16, tag="ob", bufs=4)
                        if q % 2 == 0:
                            nc.vector.tensor_copy(ob[:], ps_o[blk][q][:])
                        else:
                            nc.scalar.activation(
                                ob[:], ps_o[blk][q][:],
                                mybir.ActivationFunctionType.Copy)
                        eng = nc.sync if q % 2 == 0 else nc.scalar
                        eng.dma_start(
                            out=out_d[blk * 128:(blk + 1) * 128,
                                      q * 512:(q + 1) * 512],
                            in_=ob[:])

    nc.compile()
    return nc


_NC_CACHE = None


def _get_nc():
    global _NC_CACHE
    if _NC_CACHE is None:
        _NC_CACHE = _build()
    return _NC_CACHE


def make_in_maps(x, Wq, Wk, Wv, Wo):
    import ml_dtypes

    def conv(a):
        return np.ascontiguousarray(a).astype(ml_dtypes.bfloat16)

    x = np.asarray(x, dtype=np.float32)
    xT = conv(x.reshape(BT, C).T)                        # [C, BT] bf16

    def wlay(wT, cols):
        # [C, cols] -> [128, CT*cols]: partition p holds WT[ct*128+p, :]
        return np.ascontiguousarray(
            wT.reshape(CT, 128, cols).transpose(1, 0, 2).reshape(
                128, CT * cols))

    woT = conv(wlay(np.asarray(Wo, dtype=np.float32).T, C))  # full Wo^T
    in_maps = []
    for m in range(NCORES):
        sl = slice(m * CL, (m + 1) * CL)
        in_maps.append({
            "xt": xT,
            "wqT": conv(wlay(np.asarray(Wq)[sl, :].T, CL)),
            "wkT": conv(wlay(np.asarray(Wk)[sl, :].T, CL)),
            "wvT": conv(wlay(np.asarray(Wv)[sl, :].T, CL)),
            "woT": woT,
        })
    return in_maps


def kernel(x, Wq, Wk, Wv, Wo, _trace=False):
    in_maps = make_in_maps(x, Wq, Wk, Wv, Wo)
    nc = _get_nc()
    res = run_bass_kernel_spmd(nc, in_maps, list(range(NCORES)),
                               trace=_trace)
    out = np.concatenate([res.results[m]["out"].astype(np.float32)
                          for m in range(NCORES)], axis=0)   # [BT, C]
    out = np.ascontiguousarray(out).reshape(B, T, C)
    if _trace:
        return out, res
    return out


# revision 33
# speedup vs baseline: 56.7743x; 56.7743x over previous
"""Causal self-attention with RoPE (B=2, T=1024, C=2048, H=16) on 8 TRN2
NeuronCores, head-parallel tensor sharding (2 heads per core).
CoreSim cost-model time: ~198us/kernel (baseline AllGather design: 899us).

Design (collective-light):
  - x^T is replicated as a bf16 ExternalInput and tiles are DMA'd straight
    into SBUF at full HBM bandwidth. (The old design bounced x through
    internal DRAM + AllGather at 15us + 8MB/40GBps = 225us; direct I/O
    reads run at 360GB/s, and only DRAM->DRAM copies are slow.)
  - RoPE cos/sin tables ride as inline (Const) tensors baked into the NEFF:
    loaded to HBM at model-load time, zero runtime cost (replaces a second
    AllGather).
  - QKV projections tensor-parallel (weights column-sharded, bf16); RoPE via
    PSUM-copy-first (frees the bank for the next token block) + SBUF
    partition-shift DMA + DVE mul/add. q/k/v tiles split per batch so
    attention has no false deps on the other batch's rope writes.
  - Causal attention in [tk, tq] layout: exp on ScalarE straight out of
    PSUM, softmax denominator via an all-ones-lhsT matmul, unnormalized y
    accumulated in PSUM, one reciprocal + multiply emitting bf16 stripes.
    Fully-masked k-tiles are skipped (c0 trimming).
  - y reshard head->token via TWO per-head AllToAlls of [1024, 256] bf16
    (512KB, ~28us each) instead of two 8MB AllGathers (~450us): the h=0
    collective fires as soon as head 0 finishes and overlaps head-1
    attention; the h=1 collective overlaps the even half of the output
    projection (which only needs the h=0 AllToAll result).
  - Output projection token-parallel: this core computes ALL 2048 output
    channels for ITS 256 tokens with the full Wo (bf16, loaded during the
    QKV phase in 16 chunks on the Pool queue so the exclusive DMA resource
    is never head-of-line blocked). Output ships bf16; host upcasts.
Host reassembles: concat core outputs along the token dim.
"""
import numpy as np

import concourse.bass as bass
import concourse.mybir as mybir
import concourse.tile as tile
from concourse import bacc
from concourse.bass_utils import run_bass_kernel_spmd

F32 = mybir.dt.float32
F32R = mybir.dt.float32r
BF16 = mybir.dt.bfloat16

B, T, C = 2, 1024, 2048
H = 16
D = C // H            # 128
BT = B * T            # 2048
NCORES = 8
HL = H // NCORES      # heads per core = 2
CL = HL * D           # local channels = 256
TL = BT // NCORES     # local tokens = 256
ATT_SCALE = 1.0 / float(np.sqrt(D))
ROPE_BASE = 10000.0
NEG = -1.0e30

CT = C // 128         # 16 contraction tiles
TB = BT // 512        # 4 token blocks of 512
RG = [list(range(NCORES))]
DEBUG_DUMP = False


def _rope_tables():
    inv_freq = 1.0 / (ROPE_BASE ** (np.arange(0, D, 2, dtype=np.float64) / D))
    t = np.arange(T, dtype=np.float64)
    freqs = np.outer(t, inv_freq)                        # [T, D/2]
    emb = np.concatenate([freqs, freqs], axis=-1)        # [T, D]
    cos = np.cos(emb).astype(np.float32)                 # [T, D]
    sin = np.sin(emb).astype(np.float32)
    cosT = np.ascontiguousarray(cos.T)                   # [D, T]
    sinT = np.ascontiguousarray(sin.T)
    sgn_sinT = sinT.copy()
    sgn_sinT[: D // 2] *= -1.0                           # rotate_half sign
    return cosT, sgn_sinT


def _build():
    nc = bacc.Bacc("TRN2", target_bir_lowering=False, debug=False,
                   num_devices=NCORES)

    # full x^T, replicated on every core
    xt_d = nc.dram_tensor("xt", [C, BT], BF16, kind="ExternalInput").ap()
    # qkv weights column-sharded, laid out [128, CT*CL]: partition p holds
    # WT[ct*128+p, o] at free offset ct*CL+o
    wqT_d = nc.dram_tensor("wqT", [128, CT * CL], BF16, kind="ExternalInput").ap()
    wkT_d = nc.dram_tensor("wkT", [128, CT * CL], BF16, kind="ExternalInput").ap()
    wvT_d = nc.dram_tensor("wvT", [128, CT * CL], BF16, kind="ExternalInput").ap()
    # full Wo^T, laid out [128, CT*C]
    woT_d = nc.dram_tensor("woT", [128, CT * C], BF16, kind="ExternalInput").ap()
    # this core's 256 tokens x all 2048 output channels (bf16; host upcasts)
    out_d = nc.dram_tensor("out", [TL, C], BF16, kind="ExternalOutput").ap()
    if DEBUG_DUMP:
        dbgi_d = nc.dram_tensor("dbgi", [BT, TL], BF16,
                                kind="ExternalOutput").ap()
        dbgo_d = nc.dram_tensor("dbgo", [BT, TL], BF16,
                                kind="ExternalOutput").ap()

    # RoPE tables baked into the NEFF (loaded to HBM at model-load time)
    cosT, sgn_sinT = _rope_tables()
    cos_d = nc.inline_tensor(cosT, name="ropecos")       # [128, T] f32
    sin_d = nc.inline_tensor(sgn_sinT, name="ropesin")   # [128, T] f32

    # Per-head AllToAll buffers: [8*128 rows, 256 cols] bf16.
    #   in chunk r  = [my head-h 128 y-channels, core r's 256 tokens]
    #   out chunk s = [core s's head-h 128 y-channels, my 256 tokens]
    # Split by head so the h=0 collective hides under h=1 attention and the
    # h=1 collective hides under the even half of the output projection.
    a2a_in_d = [nc.dram_tensor(f"a2ain{h}", [NCORES * D, TL], BF16)
                for h in range(HL)]
    a2a_out_d = [nc.dram_tensor(f"a2aout{h}", [NCORES * D, TL], BF16)
                 for h in range(HL)]

    with tile.TileContext(nc) as tc:
        with (
            tc.tile_pool(name="wpool", bufs=1) as wpool,
            tc.tile_pool(name="const", bufs=1) as cpool,
            tc.tile_pool(name="qkv", bufs=1) as qkvpool,
            tc.tile_pool(name="xs", bufs=6) as xspool,
            tc.tile_pool(name="rope", bufs=1) as ropepool,
            tc.tile_pool(name="att", bufs=3) as attpool,
            tc.tile_pool(name="ya", bufs=4) as yapool,
        ):
            # ---- weight / table loads ----
            wq_sb = wpool.tile([128, CT * CL], BF16, tag="wq")
            wk_sb = wpool.tile([128, CT * CL], BF16, tag="wk")
            wv_sb = wpool.tile([128, CT * CL], BF16, tag="wv")
            nc.sync.dma_start(out=wq_sb[:], in_=wqT_d)
            nc.scalar.dma_start(out=wk_sb[:], in_=wkT_d)
            nc.gpsimd.dma_start(out=wv_sb[:], in_=wvT_d)

            cos_sb = cpool.tile([D, T], F32, tag="cos")
            sin_sb = cpool.tile([D, T], F32, tag="sin")
            nc.scalar.dma_start(out=cos_sb[:], in_=cos_d.ap())
            nc.sync.dma_start(out=sin_sb[:], in_=sin_d.ap())

            ones_f = cpool.tile([128, 128], F32, tag="onesf")
            nc.gpsimd.memset(ones_f[:], 1.0)
            ones_sb = cpool.tile([128, 128], F32R, tag="ones")
            nc.vector.tensor_copy(ones_sb[:], ones_f[:])

            # additive causal mask for diagonal 128x128 blocks:
            # rows=tk, cols=tq; keep (0.0) where tk <= tq else NEG
            mask_sb = cpool.tile([128, 128], F32, tag="mask")
            nc.gpsimd.memset(mask_sb[:], 0.0)
            nc.gpsimd.affine_select(
                out=mask_sb[:], in_=mask_sb[:],
                compare_op=mybir.AluOpType.is_ge,
                fill=NEG, base=0,
                pattern=[[1, 128]], channel_multiplier=-1,
            )

            # full Wo in SBUF (bf16, 64KB/partition), loaded in 16 chunks on
            # the Pool queue so it interleaves with the x tile stream on the
            # shared DMA resource instead of head-of-line blocking it
            wo_sb = wpool.tile([128, CT * C], BF16, tag="wo")
            for ct in range(CT):
                nc.gpsimd.dma_start(
                    out=wo_sb[:, ct * C:(ct + 1) * C],
                    in_=woT_d[:, ct * C:(ct + 1) * C])

            # persistent qkv activations, split per batch so attention on
            # batch 0 doesn't false-depend on batch-1 rope writes
            qT = [[qkvpool.tile([D, T], F32R, tag=f"qT{h}{b}", name=f"qT{h}{b}")
                   for b in range(B)] for h in range(HL)]
            kT = [[qkvpool.tile([D, T], F32R, tag=f"kT{h}{b}", name=f"kT{h}{b}")
                   for b in range(B)] for h in range(HL)]
            v_sb = [qkvpool.tile([128, (T // 128) * CL], F32R, tag=f"v{b}",
                                 name=f"v{b}")
                    for b in range(B)]

            # ---- phase 1: QKV projections + rope ----
            with tc.tile_pool(name="psqkv", bufs=1, space="PSUM") as psq:
                for tb in range(TB):
                    tcol = tb * 512
                    rcol = tcol % T          # rope table column (per batch)
                    ps_q = [psq.tile([128, 512], F32, tag=f"pq{h}", name=f"pq{h}")
                            for h in range(HL)]
                    ps_k = [psq.tile([128, 512], F32, tag=f"pk{h}", name=f"pk{h}")
                            for h in range(HL)]
                    ps_v = [psq.tile([128, CL], F32, tag=f"pv{i}", name=f"pv{i}")
                            for i in range(4)]
                    for ct in range(CT):
                        xs = xspool.tile([128, 512], BF16, tag="xs")
                        eng = nc.sync if ct % 2 == 0 else nc.scalar
                        eng.dma_start(
                            out=xs[:],
                            in_=xt_d[ct * 128:(ct + 1) * 128,
                                     tcol:tcol + 512],
                        )
                        st, sp = ct == 0, ct == CT - 1
                        for h in range(HL):
                            nc.tensor.matmul(
                                ps_q[h][:],
                                wq_sb[:, ct * CL + h * D: ct * CL + (h + 1) * D],
                                xs[:], start=st, stop=sp)
                            nc.tensor.matmul(
                                ps_k[h][:],
                                wk_sb[:, ct * CL + h * D: ct * CL + (h + 1) * D],
                                xs[:], start=st, stop=sp)
                        for i in range(4):
                            nc.tensor.matmul(
                                ps_v[i][:],
                                xs[:, i * 128:(i + 1) * 128],
                                wv_sb[:, ct * CL:(ct + 1) * CL],
                                start=st, stop=sp)
                    # rope on q, k; plain copy for v. PSUM is copied out
                    # first (one fast DVE op) so the bank frees for the next
                    # token block's matmuls; all rope math reads the copy.
                    bb = tb // 2
                    for h in range(HL):
                        for ps, dst in ((ps_q[h], qT[h][bb]),
                                        (ps_k[h], kT[h][bb])):
                            tmp = ropepool.tile([128, 512], F32, tag="rtmp",
                                                bufs=2)
                            nc.vector.tensor_copy(tmp[:], ps[:])
                            rot = ropepool.tile([128, 512], F32, tag="rrot")
                            nc.gpsimd.dma_start(out=rot[0:64, :],
                                                in_=tmp[64:128, :])
                            nc.gpsimd.dma_start(out=rot[64:128, :],
                                                in_=tmp[0:64, :])
                            t1 = ropepool.tile([128, 512], F32, tag="rt1")
                            nc.vector.tensor_mul(
                                t1[:], tmp[:], cos_sb[:, rcol:rcol + 512])
                            t2 = ropepool.tile([128, 512], F32, tag="rt2")
                            nc.vector.tensor_mul(
                                t2[:], rot[:], sin_sb[:, rcol:rcol + 512])
                            nc.vector.tensor_add(
                                dst[:, rcol:rcol + 512], t1[:], t2[:])
                    for i in range(4):
                        gt = (tb % 2) * 4 + i
                        nc.vector.tensor_copy(
                            v_sb[bb][:, gt * CL:(gt + 1) * CL], ps_v[i][:])

            # ---- phase 2: attention (h-outer); per-head AllToAll fires as
            # soon as that head's y is complete ----
            with tc.tile_pool(name="psatt", bufs=1, space="PSUM") as psa:
                for h in range(HL):
                    for b, jj in ((0, 0), (1, 0), (0, 1), (1, 1)):
                        qcol = b * T + jj * 512
                        lcol = jj * 512
                        njt = 4 * jj + 4
                        ps_y = psa.tile([128, 512], F32, tag="y",
                                        bufs=2)
                        ps_l = psa.tile([128, 512], F32, tag="l",
                                        bufs=2)
                        for j in range(njt):
                            c0 = max(0, j * 128 - jj * 512)
                            ps_s = psa.tile([128, 512], F32, tag="s",
                                            bufs=4)
                            nc.tensor.matmul(
                                ps_s[:, c0:512],
                                kT[h][b][:, j * 128:(j + 1) * 128],
                                qT[h][b][:, lcol + c0: lcol + 512],
                                start=True, stop=True)
                            diag0 = j * 128 - jj * 512
                            if 0 <= diag0 < 512:
                                nc.vector.tensor_add(
                                    ps_s[:, diag0:diag0 + 128],
                                    ps_s[:, diag0:diag0 + 128],
                                    mask_sb[:])
                            p = attpool.tile([128, 512], F32R, tag="p")
                            nc.scalar.activation(
                                p[:, c0:512], ps_s[:, c0:512],
                                mybir.ActivationFunctionType.Exp,
                                scale=ATT_SCALE)
                            st, sp = j == 0, j == njt - 1
                            nc.tensor.matmul(
                                ps_l[:, c0:512], ones_sb[:],
                                p[:, c0:512], start=st, stop=sp)
                            nc.tensor.matmul(
                                ps_y[:, c0:512],
                                v_sb[b][:, j * CL + h * D:
                                         j * CL + (h + 1) * D],
                                p[:, c0:512], start=st, stop=sp)
                        rec = attpool.tile([128, 512], F32, tag="rec")
                        nc.vector.reciprocal(rec[:], ps_l[:])
                        # normalized y straight to bf16 for the AllToAll
                        yb = attpool.tile([128, 512], BF16, tag="yb")
                        nc.vector.tensor_mul(yb[:], ps_y[:], rec[:])
                        for half in range(2):
                            r = (qcol + half * 256) // TL
                            nc.sync.dma_start(
                                out=a2a_in_d[h].ap()[r * D:(r + 1) * D, :],
                                in_=yb[:, half * 256:(half + 1) * 256])
                    # ---- phase 3 (per head): reshard y head->token ----
                    nc.gpsimd.collective_compute(
                        "AllToAll", mybir.AluOpType.bypass, replica_groups=RG,
                        ins=[a2a_in_d[h].ap()], outs=[a2a_out_d[h].ap()])
                if DEBUG_DUMP:
                    for h in range(HL):
                        nc.gpsimd.dma_start(
                            out=dbgi_d[h * NCORES * D:(h + 1) * NCORES * D, :],
                            in_=a2a_in_d[h].ap())
                        nc.gpsimd.dma_start(
                            out=dbgo_d[h * NCORES * D:(h + 1) * NCORES * D, :],
                            in_=a2a_out_d[h].ap())

            # ---- phase 4: output projection for my 256 tokens. Even-head
            # contributions (from the first AllToAll) accumulate while the
            # second AllToAll is still in flight ----
            with tc.tile_pool(name="pso", bufs=1, space="PSUM") as pso:
                ps_o = [[pso.tile([128, 512], F32, tag=f"po{blk}{q}",
                                  name=f"po{blk}{q}")
                         for q in range(4)] for blk in range(2)]
                for h in range(HL):
                    for s in range(NCORES):
                        ya = yapool.tile([128, TL], BF16, tag="ya")
                        eng = nc.sync if s % 2 == 0 else nc.scalar
                        eng.dma_start(
                            out=ya[:],
                            in_=a2a_out_d[h].ap()[s * 128:(s + 1) * 128, :])
                        ct = 2 * s + h
                        st = h == 0 and s == 0
                        sp = h == HL - 1 and s == NCORES - 1
                        for blk in range(2):
                            for q in range(4):
                                nc.tensor.matmul(
                                    ps_o[blk][q][:],
                                    ya[:, blk * 128:(blk + 1) * 128],
                                    wo_sb[:, ct * C + q * 512:
                                          ct * C + (q + 1) * 512],
                                    start=st, stop=sp)
                for blk in range(2):
                    for q in range(4):
                        ob = yapool.tile([128, 512], BF16, tag="ob", bufs=4)
                        if q % 2 == 0:
                            nc.vector.tensor_copy(ob[:], ps_o[blk][q][:])
                        else:
                            nc.scalar.activation(
                                ob[:], ps_o[blk][q][:],
                                mybir.ActivationFunctionType.Copy)
                        eng = nc.sync if q % 2 == 0 else nc.scalar
                        eng.dma_start(
                            out=out_d[blk * 128:(blk + 1) * 128,
                                      q * 512:(q + 1) * 512],
                            in_=ob[:])

    nc.compile()
    return nc


_NC_CACHE = None


def _get_nc():
    global _NC_CACHE
    if _NC_CACHE is None:
        _NC_CACHE = _build()
    return _NC_CACHE


def make_in_maps(x, Wq, Wk, Wv, Wo):
    import ml_dtypes

    def conv(a):
        return np.ascontiguousarray(a).astype(ml_dtypes.bfloat16)

    x = np.asarray(x, dtype=np.float32)
    xT = conv(x.reshape(BT, C).T)                        # [C, BT] bf16

    def wlay(wT, cols):
        # [C, cols] -> [128, CT*cols]: partition p holds WT[ct*128+p, :]
        return np.ascontiguousarray(
            wT.reshape(CT, 128, cols).transpose(1, 0, 2).reshape(
                128, CT * cols))

    woT = conv(wlay(np.asarray(Wo, dtype=np.float32).T, C))  # full Wo^T
    in_maps = []
    for m in range(NCORES):
        sl = slice(m * CL, (m + 1) * CL)
        in_maps.append({
            "xt": xT,
            "wqT": conv(wlay(np.asarray(Wq)[sl, :].T, CL)),
            "wkT": conv(wlay(np.asarray(Wk)[sl, :].T, CL)),
            "wvT": conv(wlay(np.asarray(Wv)[sl, :].T, CL)),
            "woT": woT,
        })
    return in_maps


def kernel(x, Wq, Wk, Wv, Wo, _trace=False):
    in_maps = make_in_maps(x, Wq, Wk, Wv, Wo)
    nc = _get_nc()
    res = run_bass_kernel_spmd(nc, in_maps, list(range(NCORES)),
                               trace=_trace)
    out = np.concatenate([res.results[m]["out"].astype(np.float32)
                          for m in range(NCORES)], axis=0)   # [BT, C]
    out = np.ascontiguousarray(out).reshape(B, T, C)
    if _trace:
        return out, res
    return out
